# revision 15
# baseline (speedup 1.0000x reference)
"""Trainium2 Bass kernel for NodeNetworkG GNN message passing.

Algorithm (8 NeuronCores, SPMD, no collectives):
  - Nodes are sorted by total degree and dealt round-robin to 8 cores, so each
    core owns ~1/8 of the nodes AND ~1/8 of the edges for both aggregation
    directions (mi: group by col, mo: group by row).
  - Per core, owned destinations are packed into 128-node blocks; each block
    gets S slot-columns (S = max degree in block, rounded up to even). Edges
    are laid out host-side as [128, ncols] int32 gather-index / f32 weight
    arrays; padded slots point at a zeros row of the x table with weight 0.
  - Device: indirect-DMA gathers x rows per slot, DVE multiplies by edge
    weight, then a pairwise fold tree sums the S slots per destination,
    yielding mi/mo [128, nb*48] resident in SBUF.
  - Node-wise MLP: PE transposes [mi|mo|x] tiles to feature-major, two matmuls
    with tanh activations (bias via ACT per-partition bias), output written
    transposed; host restores order.
"""

import math

import numpy as np

P = 128
NCORES = 8
DIN = 48
DHID = 128

_PROG_CACHE: dict = {}


def _round_up(a, m):
    return (a + m - 1) // m * m


def _slot_layout(S_blk):
    """Given per-block slot counts (true block order), build the direction
    layout: blocks sorted by S desc; returns (ord_blocks, pos, col_off, total_cols,
    groups) where groups = list of (S, nbs, col_base, out_base_blkpos) runs of
    equal S split later into subchunks."""
    nb = len(S_blk)
    ordb = sorted(range(nb), key=lambda b: -S_blk[b])
    pos = [0] * nb
    for j, b in enumerate(ordb):
        pos[b] = j
    col_off = [0] * nb  # column offset (in slot units) of j-th block in ordb
    total = 0
    for j, b in enumerate(ordb):
        col_off[j] = total
        total += S_blk[b]
    # group runs of equal S (in ordb order)
    groups = []
    j = 0
    while j < nb:
        S = S_blk[ordb[j]]
        j0 = j
        while j < nb and S_blk[ordb[j]] == S:
            j += 1
        groups.append((S, j0, j - j0))  # S, first block pos, n blocks
    return ordb, pos, col_off, total, groups


def _host_prep(x, edge_index, edge_attr):
    N = x.shape[0]
    row = np.asarray(edge_index[0]).astype(np.int32)
    col = np.asarray(edge_index[1]).astype(np.int32)
    w = np.asarray(edge_attr, dtype=np.float32).reshape(-1)
    E = row.shape[0]

    npad = _round_up(N, P * NCORES)
    nloc = npad // NCORES
    nb = nloc // P

    deg_in = np.bincount(col, minlength=npad)
    deg_out = np.bincount(row, minlength=npad)
    order = np.argsort(-(deg_in + deg_out), kind="stable")
    rank = np.empty(npad, np.int64)
    rank[order] = np.arange(npad)
    core = (rank % NCORES).astype(np.int32)
    local = (rank // NCORES).astype(np.int64)
    blk = (local // P).astype(np.int32)
    part = (local % P).astype(np.int32)

    # unified per-block slot counts (max over cores), rounded up to even
    def blk_S(deg):
        m = np.zeros(nb, np.int64)
        np.maximum.at(m, blk, deg)
        m = np.where(m > 0, _round_up_arr(m, 2), 0)
        return m

    S_in = blk_S(deg_in)
    S_out = blk_S(deg_out)

    lay_in = _slot_layout(list(S_in))
    lay_out = _slot_layout(list(S_out))
    CM = lay_in[3]
    CO = lay_out[3]

    ZROW = N  # zeros row index in x_tab

    def build_dir(dest, src, lay):
        """idx/w arrays [NCORES, P, C] for one direction."""
        _ordb, pos, col_off, C, _groups = lay
        idx = np.full((NCORES, P, C), ZROW, np.int32)
        wv = np.zeros((NCORES, P, C), np.float32)
        # slot rank within destination
        sortp = np.argsort(dest, kind="stable")
        dsort = dest[sortp]
        deg = np.bincount(dest, minlength=npad)
        first = np.cumsum(deg) - deg
        srank = np.arange(E) - first[dsort]
        e = sortp  # original edge ids in dest-sorted order
        d = dsort
        c_e = core[d]
        b_e = blk[d]
        p_e = part[d]
        colpos = np.asarray(col_off, np.int64)
        posarr = np.asarray(pos, np.int64)
        cidx = colpos[posarr[b_e]] + srank
        idx[c_e, p_e, cidx] = src[e]
        wv[c_e, p_e, cidx] = w[e]
        return idx, wv

    idx_mi, w_mi = build_dir(col, row, lay_in)
    idx_mo, w_mo = build_dir(row, col, lay_out)

    # x table with zeros row
    x_tab = np.zeros((N + 1, DIN), np.float32)
    x_tab[:N] = np.asarray(x, np.float32)

    # own-node x gather indices [NCORES, P, nb]
    idx_own = np.full((NCORES, P, nb), ZROW, np.int32)
    nodes = np.arange(npad)
    valid = nodes < N
    idx_own[core[nodes[valid]], part[nodes[valid]], blk[nodes[valid]]] = nodes[
        valid
    ].astype(np.int32)

    nodes_of_core = [order[k::NCORES] for k in range(NCORES)]

    meta = dict(
        N=N,
        E=E,
        npad=npad,
        nb=nb,
        CM=CM,
        CO=CO,
        lay_in=lay_in,
        lay_out=lay_out,
        x_tab=x_tab,
        idx_mi=idx_mi,
        w_mi=w_mi,
        idx_mo=idx_mo,
        w_mo=w_mo,
        idx_own=idx_own,
        nodes_of_core=nodes_of_core,
    )
    return meta


def _round_up_arr(a, m):
    return (a + m - 1) // m * m


def _posoff(lay):
    """col_off indexed by j (position in ordb)."""
    return lay[2]


def _subchunks(lay, CC):
    """Split the direction layout into subchunks:
    (S, nbs, col_base_slots, out_blkpos) with S*nbs <= CC."""
    _ordb, _pos, col_off, _total, groups = lay
    out = []
    for S, j0, njb in groups:
        if S == 0:
            continue
        nbs_max = max(1, CC // S)
        j = 0
        while j < njb:
            nbs = min(nbs_max, njb - j)
            out.append((S, nbs, col_off[j0 + j], j0 + j))
            j += nbs
    return out


def _build_program(meta, CC=96):
    import concourse.bacc as bacc
    import concourse.bass as bass
    import concourse.mybir as mybir
    import concourse.tile as tile
    from concourse.masks import make_identity

    N = meta["N"]
    nb = meta["nb"]
    CM = meta["CM"]
    CO = meta["CO"]
    f32 = mybir.dt.float32
    i32 = mybir.dt.int32

    nc = bacc.Bacc("TRN2", target_bir_lowering=False, debug=False, num_devices=NCORES)

    x_tab = nc.dram_tensor("x_tab", [N + 1, DIN], f32, kind="ExternalInput")
    idx_mi_d = nc.dram_tensor("idx_mi", [P, CM], i32, kind="ExternalInput")
    w_mi_d = nc.dram_tensor("w_mi", [P, CM], f32, kind="ExternalInput")
    idx_mo_d = nc.dram_tensor("idx_mo", [P, CO], i32, kind="ExternalInput")
    w_mo_d = nc.dram_tensor("w_mo", [P, CO], f32, kind="ExternalInput")
    idx_own_d = nc.dram_tensor("idx_own", [P, nb], i32, kind="ExternalInput")
    w1ta_d = nc.dram_tensor("w1ta", [DIN, DHID], f32, kind="ExternalInput")
    w1tb_d = nc.dram_tensor("w1tb", [DIN, DHID], f32, kind="ExternalInput")
    w1tc_d = nc.dram_tensor("w1tc", [DIN, DHID], f32, kind="ExternalInput")
    w2t_d = nc.dram_tensor("w2t", [DHID, DHID], f32, kind="ExternalInput")
    b1_d = nc.dram_tensor("b1", [DHID, 1], f32, kind="ExternalInput")
    b2_d = nc.dram_tensor("b2", [DHID, 1], f32, kind="ExternalInput")
    out_t = nc.dram_tensor("out_t", [P, nb * P], f32, kind="ExternalOutput")

    with tile.TileContext(nc) as tc:
        with (
            tc.tile_pool(name="const", bufs=1) as const,
            tc.tile_pool(name="gpool", bufs=2) as gpool,
            tc.tile_pool(name="mlp", bufs=3) as mlp,
            tc.tile_pool(name="ost", bufs=2) as ostp,
            tc.tile_pool(name="psA", bufs=3, space="PSUM") as psA,
            tc.tile_pool(name="psH", bufs=2, space="PSUM") as psH,
        ):
            # ---- load constants / index arrays ----
            idx_mi_sb = const.tile([P, CM], i32)
            nc.sync.dma_start(idx_mi_sb[:], idx_mi_d[:])
            w_mi_sb = const.tile([P, CM], f32)
            nc.sync.dma_start(w_mi_sb[:], w_mi_d[:])
            idx_mo_sb = const.tile([P, CO], i32)
            nc.sync.dma_start(idx_mo_sb[:], idx_mo_d[:])
            w_mo_sb = const.tile([P, CO], f32)
            nc.sync.dma_start(w_mo_sb[:], w_mo_d[:])
            idx_own_sb = const.tile([P, nb], i32)
            nc.sync.dma_start(idx_own_sb[:], idx_own_d[:])
            w1ta_sb = const.tile([DIN, DHID], f32)
            nc.sync.dma_start(w1ta_sb[:], w1ta_d[:])
            w1tb_sb = const.tile([DIN, DHID], f32)
            nc.sync.dma_start(w1tb_sb[:], w1tb_d[:])
            w1tc_sb = const.tile([DIN, DHID], f32)
            nc.sync.dma_start(w1tc_sb[:], w1tc_d[:])
            w2t_sb = const.tile([DHID, DHID], f32)
            nc.sync.dma_start(w2t_sb[:], w2t_d[:])
            b1_sb = const.tile([DHID, 1], f32)
            nc.sync.dma_start(b1_sb[:], b1_d[:])
            b2_sb = const.tile([DHID, 1], f32)
            nc.sync.dma_start(b2_sb[:], b2_d[:])
            ident = const.tile([P, P], f32)
            make_identity(nc, ident[:])

            mi_sb = const.tile([P, nb * DIN], f32)
            mo_sb = const.tile([P, nb * DIN], f32)
            xo_sb = const.tile([P, nb * DIN], f32)

            # own-x gather straight into xo_sb (HW: one index per partition
            # per indirect DMA -> one instruction per slot column)
            for c in range(nb):
                nc.gpsimd.indirect_dma_start(
                    out=xo_sb[:, c * DIN : (c + 1) * DIN],
                    out_offset=None,
                    in_=x_tab[:],
                    in_offset=bass.IndirectOffsetOnAxis(
                        ap=idx_own_sb[:, c : c + 1], axis=0
                    ),
                )

            # ---- aggregation ----
            for lay, idx_sb, w_sb, acc_sb in (
                (meta["lay_in"], idx_mi_sb, w_mi_sb, mi_sb),
                (meta["lay_out"], idx_mo_sb, w_mo_sb, mo_sb),
            ):
                # zero-degree tail blocks: memset their accumulator columns
                zblocks = [j for j, b in enumerate(lay[0]) if _S_of(lay, j) == 0]
                if zblocks:
                    z0 = min(zblocks)
                    nzb = len(zblocks)
                    nc.vector.memset(acc_sb[:, z0 * DIN : (z0 + nzb) * DIN], 0.0)
                for S, nbs, c0, outpos in _subchunks(lay, CC):
                    cols = S * nbs
                    G = gpool.tile([P, CC * DIN], f32, tag="G")
                    g3 = G[:, : cols * DIN].rearrange("p (c f) -> p c f", f=DIN)
                    for c in range(cols):
                        nc.gpsimd.indirect_dma_start(
                            out=G[:, c * DIN : (c + 1) * DIN],
                            out_offset=None,
                            in_=x_tab[:],
                            in_offset=bass.IndirectOffsetOnAxis(
                                ap=idx_sb[:, c0 + c : c0 + c + 1], axis=0
                            ),
                        )
                    # multiply by per-slot weight (broadcast over feature dim)
                    wv = w_sb[:, c0 : c0 + cols]
                    wb = bass.AP(
                        wv.tensor,
                        wv.offset,
                        [list(wv.ap[0]), list(wv.ap[1]), [0, DIN]],
                    )
                    nc.vector.tensor_tensor(
                        out=g3, in0=g3, in1=wb, op=mybir.AluOpType.mult
                    )
                    # fold S slots -> slot 0 (pairwise tree), per block
                    g4 = G[:, : cols * DIN].rearrange(
                        "p (b s f) -> p b s f", s=S, f=DIN
                    )
                    s = S
                    while s > 1:
                        half = s // 2
                        hi0 = s - half
                        nc.vector.tensor_tensor(
                            out=g4[:, :, 0:half, :],
                            in0=g4[:, :, 0:half, :],
                            in1=g4[:, :, hi0:s, :],
                            op=mybir.AluOpType.add,
                        )
                        s = hi0
                    # copy folded result to accumulator columns
                    nc.vector.tensor_copy(
                        out=acc_sb[:, outpos * DIN : (outpos + nbs) * DIN].rearrange(
                            "p (b f) -> p b f", f=DIN
                        ),
                        in_=g4[:, :, 0, :],
                    )

            # ---- MLP over 128-node tiles ----
            pos_in = meta["lay_in"][1]
            pos_out = meta["lay_out"][1]
            OG = 4  # output tiles per DMA group
            for b0 in range(0, nb, OG):
                og = min(OG, nb - b0)
                os_ = ostp.tile([P, OG * P], f32, tag="os")
                for j in range(og):
                    b = b0 + j
                    mi_c = mi_sb[:, pos_in[b] * DIN : (pos_in[b] + 1) * DIN]
                    mo_c = mo_sb[:, pos_out[b] * DIN : (pos_out[b] + 1) * DIN]
                    xo_c = xo_sb[:, b * DIN : (b + 1) * DIN]
                    hp = psH.tile([P, P], f32, tag="hp")
                    for q, (src_c, w1q) in enumerate(
                        ((mi_c, w1ta_sb), (mo_c, w1tb_sb), (xo_c, w1tc_sb))
                    ):
                        pA = psA.tile([DIN, P], f32, tag="pA")
                        nc.tensor.transpose(pA[:], src_c, ident[:])
                        mt = mlp.tile([DIN, P], f32, tag="mt")
                        nc.vector.tensor_copy(out=mt[:], in_=pA[:])
                        nc.tensor.matmul(
                            hp[:], w1q[:], mt[:], start=(q == 0), stop=(q == 2)
                        )
                    hs = mlp.tile([P, P], f32, tag="hs")
                    nc.scalar.activation(
                        hs[:],
                        hp[:],
                        mybir.ActivationFunctionType.Tanh,
                        bias=b1_sb[:],
                        scale=1.0,
                    )
                    op_ = psH.tile([P, P], f32, tag="op")
                    nc.tensor.matmul(op_[:], w2t_sb[:], hs[:], start=True, stop=True)
                    nc.scalar.activation(
                        os_[:, j * P : (j + 1) * P],
                        op_[:],
                        mybir.ActivationFunctionType.Tanh,
                        bias=b2_sb[:],
                        scale=1.0,
                    )
                nc.sync.dma_start(
                    out_t[:, b0 * P : (b0 + og) * P], os_[:, : og * P]
                )

    nc.compile()
    return nc


def _S_of(lay, j):
    """Slot count of j-th block (in ordb order)."""
    ordb = lay[0]
    col_off = lay[2]
    total = lay[3]
    nxt = col_off[j + 1] if j + 1 < len(col_off) else total
    return nxt - col_off[j]


def kernel(x, edge_index, edge_attr, W1, b1, W2, b2):
    x = np.asarray(x, np.float32)
    meta = _host_prep(x, edge_index, edge_attr)
    key = (meta["N"], meta["E"], meta["nb"], meta["CM"], meta["CO"],
           tuple(meta["lay_in"][2]), tuple(meta["lay_out"][2]))
    if key not in _PROG_CACHE:
        _PROG_CACHE[key] = _build_program(meta)
    nc = _PROG_CACHE[key]

    W1 = np.asarray(W1, np.float32)
    W2 = np.asarray(W2, np.float32)
    b1 = np.asarray(b1, np.float32).reshape(DHID, 1)
    b2 = np.asarray(b2, np.float32).reshape(DHID, 1)
    w1t = np.ascontiguousarray(W1.T)  # [144, 128]
    w1ta = np.ascontiguousarray(w1t[:DIN])
    w1tb = np.ascontiguousarray(w1t[DIN : 2 * DIN])
    w1tc = np.ascontiguousarray(w1t[2 * DIN :])
    w2t = np.ascontiguousarray(W2.T)

    in_maps = []
    for k in range(NCORES):
        in_maps.append(
            {
                "x_tab": meta["x_tab"],
                "idx_mi": meta["idx_mi"][k],
                "w_mi": meta["w_mi"][k],
                "idx_mo": meta["idx_mo"][k],
                "w_mo": meta["w_mo"][k],
                "idx_own": meta["idx_own"][k],
                "w1ta": w1ta,
                "w1tb": w1tb,
                "w1tc": w1tc,
                "w2t": w2t,
                "b1": b1,
                "b2": b2,
            }
        )

    runner = _get_runner(nc)
    results = runner.run(in_maps)
    global _LAST
    _LAST = (nc, in_maps)

    N = meta["N"]
    out = np.empty((meta["npad"], DHID), np.float32)
    for k in range(NCORES):
        out[meta["nodes_of_core"][k]] = results[k]["out_t"].T
    return out[:N]


_LAST = None
_RUNNER_CACHE: dict = {}


class _PjrtRunner:
    """Builds the shard_map-jitted NEFF executor once; supports repeated
    dispatches with device-resident inputs for timing."""

    def __init__(self, nc):
        import jax
        import jax.numpy as jnp
        import concourse.mybir as mybir
        from concourse import bass2jax
        from jax.sharding import Mesh, NamedSharding, PartitionSpec
        from jax.experimental.shard_map import shard_map

        bass2jax.install_neuronx_cc_hook()
        self.jax = jax
        self.jnp = jnp
        in_names: list[str] = []
        out_names: list[str] = []
        out_avals = []
        out_shapes = []
        partition_name = (
            nc.partition_id_tensor.name if nc.partition_id_tensor else None
        )
        for alloc in nc.m.functions[0].allocations:
            if not isinstance(alloc, mybir.MemoryLocationSet):
                continue
            name = alloc.memorylocations[0].name
            if alloc.kind == "ExternalInput":
                if name != partition_name:
                    in_names.append(name)
            elif alloc.kind == "ExternalOutput":
                shape = tuple(alloc.tensor_shape)
                dtype = mybir.dt.np(alloc.dtype)
                out_names.append(name)
                out_avals.append(jax.core.ShapedArray(shape, dtype))
                out_shapes.append((shape, dtype))
        self.in_names = in_names
        self.out_names = out_names
        self.out_shapes = out_shapes
        n_params = len(in_names)
        n_outs = len(out_names)
        all_names = in_names + out_names
        if partition_name is not None:
            all_names = all_names + [partition_name]

        def _body(*args):
            operands = list(args)
            if partition_name is not None:
                operands.append(bass2jax.partition_id_tensor())
            outs = bass2jax._bass_exec_p.bind(
                *operands,
                out_avals=tuple(out_avals),
                in_names=tuple(all_names),
                out_names=tuple(out_names),
                lowering_input_output_aliases=(),
                sim_require_finite=True,
                sim_require_nnan=True,
                nc=nc,
            )
            return tuple(outs)

        devices = jax.devices()[:NCORES]
        self.mesh = Mesh(np.asarray(devices), ("core",))
        spec = PartitionSpec("core")
        self.sharding = NamedSharding(self.mesh, spec)
        self.sharded = jax.jit(
            shard_map(
                _body,
                mesh=self.mesh,
                in_specs=(spec,) * (n_params + n_outs),
                out_specs=(spec,) * n_outs,
                check_rep=False,
            ),
            donate_argnums=tuple(range(n_params, n_params + n_outs)),
            keep_unused=True,
        )

        def _mk_zeros():
            return tuple(
                jnp.zeros((NCORES * s[0], *s[1:]), d) for s, d in out_shapes
            )

        self.zeros_fn = jax.jit(
            _mk_zeros, out_shardings=(self.sharding,) * n_outs
        )

    def _stage_inputs(self, in_maps):
        concat = [
            np.concatenate([np.asarray(in_maps[c][n]) for c in range(NCORES)], axis=0)
            for n in self.in_names
        ]
        return [self.jax.device_put(a, self.sharding) for a in concat]

    def _dispatch(self, staged):
        zeros = self.zeros_fn()
        outs = self.sharded(*staged, *zeros)
        self.jax.block_until_ready(outs)
        return outs

    def run(self, in_maps):
        staged = self._stage_inputs(in_maps)
        outs = self._dispatch(staged)
        res = []
        for c in range(NCORES):
            m = {}
            for i, n in enumerate(self.out_names):
                s, d = self.out_shapes[i]
                m[n] = np.asarray(outs[i]).reshape(NCORES, *s)[c]
            res.append(m)
        return res

    def timed(self, in_maps, iters=8):
        import time

        staged = self._stage_inputs(in_maps)
        self._dispatch(staged)  # warm
        walls = []
        for _ in range(iters):
            zeros = self.zeros_fn()
            self.jax.block_until_ready(zeros)
            t0 = time.perf_counter()
            outs = self.sharded(*staged, *zeros)
            self.jax.block_until_ready(outs)
            walls.append(time.perf_counter() - t0)
        # dispatch-overhead baseline: trivial jitted op on tiny sharded array
        jnp = self.jnp
        tiny = self.jax.device_put(
            np.zeros((NCORES, 8), np.float32), self.sharding
        )
        base_fn = self.jax.jit(lambda a: a + 1.0)
        self.jax.block_until_ready(base_fn(tiny))
        bases = []
        for _ in range(iters):
            t0 = time.perf_counter()
            self.jax.block_until_ready(base_fn(tiny))
            bases.append(time.perf_counter() - t0)
        wall = float(np.median(walls))
        base = float(np.median(bases))
        print(
            f"wall/call: {[f'{w*1e3:.2f}ms' for w in walls]}  "
            f"baseline: {base*1e3:.2f}ms"
        )
        return max(wall - base, 0.0) * 1e9


def _get_runner(nc):
    r = _RUNNER_CACHE.get(id(nc))
    if r is None:
        r = _PjrtRunner(nc)
        _RUNNER_CACHE[id(nc)] = r
    return r


def time_kernel(inputs=None, iters=8):
    """Median wall-clock of a kernel dispatch minus tiny-dispatch baseline, ns.
    Must be called after kernel()."""
    assert _LAST is not None, "call kernel() first"
    nc, in_maps = _LAST
    return _get_runner(nc).timed(in_maps, iters=iters)


# revision 24
# speedup vs baseline: 3.1746x; 3.1746x over previous
"""Trainium2 Bass kernel for NodeNetworkG GNN message passing.

Algorithm (8 NeuronCores, SPMD, no collectives):
  - Nodes are sorted by total degree and dealt round-robin to 8 cores, so each
    core owns ~1/8 of the nodes AND ~1/8 of the edges for both aggregation
    directions (mi: group by col, mo: group by row).
  - Per core, owned destinations are packed into 128-node blocks; each block
    gets S slot-columns (S = max degree in block, rounded up to even). Edges
    are laid out host-side as [128, ncols] int32 gather-index / f32 weight
    arrays; padded slots point at a zeros row of the x table with weight 0.
  - Device: indirect-DMA gathers x rows per slot, DVE multiplies by edge
    weight, then a pairwise fold tree sums the S slots per destination,
    yielding mi/mo [128, nb*48] resident in SBUF.
  - Node-wise MLP: PE transposes [mi|mo|x] tiles to feature-major, two matmuls
    with tanh activations (bias via ACT per-partition bias), output written
    transposed; host restores order.
"""

import math

import numpy as np

P = 128
NCORES = 8
DIN = 48
DHID = 128

_PROG_CACHE: dict = {}


def _round_up(a, m):
    return (a + m - 1) // m * m


def _slot_layout(S_blk):
    """Given per-block slot counts (true block order), build the direction
    layout: blocks sorted by S desc; returns (ord_blocks, pos, col_off, total_cols,
    groups) where groups = list of (S, nbs, col_base, out_base_blkpos) runs of
    equal S split later into subchunks."""
    nb = len(S_blk)
    ordb = sorted(range(nb), key=lambda b: -S_blk[b])
    pos = [0] * nb
    for j, b in enumerate(ordb):
        pos[b] = j
    col_off = [0] * nb  # column offset (in slot units) of j-th block in ordb
    total = 0
    for j, b in enumerate(ordb):
        col_off[j] = total
        total += S_blk[b]
    # group runs of equal S (in ordb order)
    groups = []
    j = 0
    while j < nb:
        S = S_blk[ordb[j]]
        j0 = j
        while j < nb and S_blk[ordb[j]] == S:
            j += 1
        groups.append((S, j0, j - j0))  # S, first block pos, n blocks
    return ordb, pos, col_off, total, groups


def _host_prep(x, edge_index, edge_attr):
    N = x.shape[0]
    row = np.asarray(edge_index[0]).astype(np.int32)
    col = np.asarray(edge_index[1]).astype(np.int32)
    w = np.asarray(edge_attr, dtype=np.float32).reshape(-1)
    E = row.shape[0]

    npad = _round_up(N, P * NCORES)
    nloc = npad // NCORES
    nb = nloc // P

    deg_in = np.bincount(col, minlength=npad)
    deg_out = np.bincount(row, minlength=npad)
    order = np.argsort(-(deg_in + deg_out), kind="stable")
    rank = np.empty(npad, np.int64)
    rank[order] = np.arange(npad)
    core = (rank % NCORES).astype(np.int32)

    def direction_maps(deg):
        """Per-core block packing sorted by this direction's degree.
        Returns blk[node], part[node], S_b (unified over cores)."""
        blk = np.empty(npad, np.int32)
        part = np.empty(npad, np.int32)
        for k in range(NCORES):
            nodes_k = np.where(core == k)[0]
            lk = nodes_k[np.argsort(-deg[nodes_k], kind="stable")]
            pos = np.arange(nloc)
            blk[lk] = (pos // P).astype(np.int32)
            part[lk] = (pos % P).astype(np.int32)
        m = np.zeros(nb, np.int64)
        np.maximum.at(m, blk, deg)
        S = np.where(m > 0, _round_up_arr(m, 2), 0)
        return blk, part, S

    blk_i, part_i, S_in = direction_maps(deg_in)
    blk_o, part_o, S_out = direction_maps(deg_out)

    lay_in = _slot_layout(list(S_in))
    lay_out = _slot_layout(list(S_out))
    CM = lay_in[3]
    CO = lay_out[3]

    ZROW = N  # zeros row index in x_tab

    def build_dir(dest, src, lay, blk, part):
        """idx/w arrays [NCORES, P, C] for one direction."""
        _ordb, pos, col_off, C, _groups = lay
        idx = np.full((NCORES, P, C), ZROW, np.int32)
        wv = np.zeros((NCORES, P, C), np.float32)
        # slot rank within destination
        sortp = np.argsort(dest, kind="stable")
        dsort = dest[sortp]
        deg = np.bincount(dest, minlength=npad)
        first = np.cumsum(deg) - deg
        srank = np.arange(E) - first[dsort]
        e = sortp  # original edge ids in dest-sorted order
        d = dsort
        c_e = core[d]
        b_e = blk[d]
        p_e = part[d]
        colpos = np.asarray(col_off, np.int64)
        posarr = np.asarray(pos, np.int64)
        cidx = colpos[posarr[b_e]] + srank
        idx[c_e, p_e, cidx] = src[e]
        wv[c_e, p_e, cidx] = w[e]
        return idx, wv

    idx_mi, w_mi = build_dir(col, row, lay_in, blk_i, part_i)
    idx_mo, w_mo = build_dir(row, col, lay_out, blk_o, part_o)

    # x table with zeros row
    x_tab = np.zeros((N + 1, DIN), np.float32)
    x_tab[:N] = np.asarray(x, np.float32)

    # own-node x gather indices [NCORES, P, nb] in mi-local order
    idx_own = np.full((NCORES, P, nb), ZROW, np.int32)
    nodes = np.arange(npad)
    valid = nodes < N
    idx_own[core[nodes[valid]], part_i[nodes[valid]], blk_i[nodes[valid]]] = nodes[
        valid
    ].astype(np.int32)

    # realign indices: for mi-local slot (c=blk_i, p=part_i) of node n, the row
    # of n in the mo scratch layout (row = pos_out[blk_o]*128 + part_o)
    pos_out = np.asarray(lay_out[1], np.int64)
    realign = np.zeros((NCORES, P, nb), np.int32)
    realign[core, part_i, blk_i] = (pos_out[blk_o] * P + part_o).astype(np.int32)

    # output row mapping: out_t column (b*128+p) of core k = node at mi-local
    nodes_of_core = []
    for k in range(NCORES):
        lk = np.full(nloc, N, np.int64)  # dummies discarded later
        sel = core == k
        lk[blk_i[sel].astype(np.int64) * P + part_i[sel]] = np.where(sel)[0]
        nodes_of_core.append(lk)

    meta = dict(
        N=N,
        E=E,
        npad=npad,
        nb=nb,
        CM=CM,
        CO=CO,
        lay_in=lay_in,
        lay_out=lay_out,
        x_tab=x_tab,
        idx_mi=idx_mi,
        w_mi=w_mi,
        idx_mo=idx_mo,
        w_mo=w_mo,
        idx_own=idx_own,
        realign=realign,
        nodes_of_core=nodes_of_core,
    )
    return meta


def _round_up_arr(a, m):
    return (a + m - 1) // m * m


def _posoff(lay):
    """col_off indexed by j (position in ordb)."""
    return lay[2]


def _subchunks(lay, CC):
    """Split the direction layout into subchunks:
    (S, nbs, col_base_slots, out_blkpos) with S*nbs <= CC."""
    _ordb, _pos, col_off, _total, groups = lay
    out = []
    for S, j0, njb in groups:
        if S == 0:
            continue
        nbs_max = max(1, CC // S)
        j = 0
        while j < njb:
            nbs = min(nbs_max, njb - j)
            out.append((S, nbs, col_off[j0 + j], j0 + j))
            j += nbs
    return out


def _build_program(meta, CC=96):
    import concourse.bacc as bacc
    import concourse.bass as bass
    import concourse.mybir as mybir
    import concourse.tile as tile
    from concourse.masks import make_identity

    N = meta["N"]
    nb = meta["nb"]
    CM = meta["CM"]
    CO = meta["CO"]
    f32 = mybir.dt.float32
    i32 = mybir.dt.int32

    nc = bacc.Bacc("TRN2", target_bir_lowering=False, debug=False, num_devices=NCORES)

    x_tab = nc.dram_tensor("x_tab", [N + 1, DIN], f32, kind="ExternalInput")
    idx_mi_d = nc.dram_tensor("idx_mi", [P, CM], i32, kind="ExternalInput")
    w_mi_d = nc.dram_tensor("w_mi", [P, CM], f32, kind="ExternalInput")
    idx_mo_d = nc.dram_tensor("idx_mo", [P, CO], i32, kind="ExternalInput")
    w_mo_d = nc.dram_tensor("w_mo", [P, CO], f32, kind="ExternalInput")
    idx_own_d = nc.dram_tensor("idx_own", [P, nb], i32, kind="ExternalInput")
    realign_d = nc.dram_tensor("realign", [P, nb], i32, kind="ExternalInput")
    mo_scratch = nc.dram_tensor("mo_scratch", [nb * P, DIN], f32, kind="Internal")
    w1ta_d = nc.dram_tensor("w1ta", [DIN, DHID], f32, kind="ExternalInput")
    w1tb_d = nc.dram_tensor("w1tb", [DIN, DHID], f32, kind="ExternalInput")
    w1tc_d = nc.dram_tensor("w1tc", [DIN, DHID], f32, kind="ExternalInput")
    w2t_d = nc.dram_tensor("w2t", [DHID, DHID], f32, kind="ExternalInput")
    b1_d = nc.dram_tensor("b1", [DHID, 1], f32, kind="ExternalInput")
    b2_d = nc.dram_tensor("b2", [DHID, 1], f32, kind="ExternalInput")
    out_t = nc.dram_tensor("out_t", [P, nb * P], f32, kind="ExternalOutput")

    with tile.TileContext(nc) as tc:
        with (
            tc.tile_pool(name="const", bufs=1) as const,
            tc.tile_pool(name="gpool", bufs=2) as gpool,
            tc.tile_pool(name="mlp", bufs=3) as mlp,
            tc.tile_pool(name="ost", bufs=2) as ostp,
            tc.tile_pool(name="psA", bufs=3, space="PSUM") as psA,
            tc.tile_pool(name="psH", bufs=2, space="PSUM") as psH,
        ):
            # ---- load constants / index arrays ----
            idx_mi_sb = const.tile([P, CM], i32)
            nc.sync.dma_start(idx_mi_sb[:], idx_mi_d[:])
            w_mi_sb = const.tile([P, CM], f32)
            nc.sync.dma_start(w_mi_sb[:], w_mi_d[:])
            idx_mo_sb = const.tile([P, CO], i32)
            nc.sync.dma_start(idx_mo_sb[:], idx_mo_d[:])
            w_mo_sb = const.tile([P, CO], f32)
            nc.sync.dma_start(w_mo_sb[:], w_mo_d[:])
            idx_own_sb = const.tile([P, nb], i32)
            nc.sync.dma_start(idx_own_sb[:], idx_own_d[:])
            realign_sb = const.tile([P, nb], i32)
            nc.sync.dma_start(realign_sb[:], realign_d[:])
            w1ta_sb = const.tile([DIN, DHID], f32)
            nc.sync.dma_start(w1ta_sb[:], w1ta_d[:])
            w1tb_sb = const.tile([DIN, DHID], f32)
            nc.sync.dma_start(w1tb_sb[:], w1tb_d[:])
            w1tc_sb = const.tile([DIN, DHID], f32)
            nc.sync.dma_start(w1tc_sb[:], w1tc_d[:])
            w2t_sb = const.tile([DHID, DHID], f32)
            nc.sync.dma_start(w2t_sb[:], w2t_d[:])
            b1_sb = const.tile([DHID, 1], f32)
            nc.sync.dma_start(b1_sb[:], b1_d[:])
            b2_sb = const.tile([DHID, 1], f32)
            nc.sync.dma_start(b2_sb[:], b2_d[:])
            ident = const.tile([P, P], f32)
            make_identity(nc, ident[:])

            mi_sb = const.tile([P, nb * DIN], f32)
            mo_sb = const.tile([P, nb * DIN], f32)
            mo2_sb = const.tile([P, nb * DIN], f32)
            xo_sb = const.tile([P, nb * DIN], f32)

            # own-x gather straight into xo_sb (HW: one index per partition
            # per indirect DMA -> one instruction per slot column)
            for c in range(nb):
                nc.gpsimd.indirect_dma_start(
                    out=xo_sb[:, c * DIN : (c + 1) * DIN],
                    out_offset=None,
                    in_=x_tab[:],
                    in_offset=bass.IndirectOffsetOnAxis(
                        ap=idx_own_sb[:, c : c + 1], axis=0
                    ),
                )

            # ---- aggregation ----
            for lay, idx_sb, w_sb, acc_sb in (
                (meta["lay_in"], idx_mi_sb, w_mi_sb, mi_sb),
                (meta["lay_out"], idx_mo_sb, w_mo_sb, mo_sb),
            ):
                # zero-degree tail blocks: memset their accumulator columns
                zblocks = [j for j, b in enumerate(lay[0]) if _S_of(lay, j) == 0]
                if zblocks:
                    z0 = min(zblocks)
                    nzb = len(zblocks)
                    nc.vector.memset(acc_sb[:, z0 * DIN : (z0 + nzb) * DIN], 0.0)
                for S, nbs, c0, outpos in _subchunks(lay, CC):
                    cols = S * nbs
                    G = gpool.tile([P, CC * DIN], f32, tag="G")
                    g3 = G[:, : cols * DIN].rearrange("p (c f) -> p c f", f=DIN)
                    for c in range(cols):
                        nc.gpsimd.indirect_dma_start(
                            out=G[:, c * DIN : (c + 1) * DIN],
                            out_offset=None,
                            in_=x_tab[:],
                            in_offset=bass.IndirectOffsetOnAxis(
                                ap=idx_sb[:, c0 + c : c0 + c + 1], axis=0
                            ),
                        )
                    # multiply by per-slot weight (broadcast over feature dim)
                    wv = w_sb[:, c0 : c0 + cols]
                    wb = bass.AP(
                        wv.tensor,
                        wv.offset,
                        [list(wv.ap[0]), list(wv.ap[1]), [0, DIN]],
                    )
                    nc.vector.tensor_tensor(
                        out=g3, in0=g3, in1=wb, op=mybir.AluOpType.mult
                    )
                    # fold S slots -> slot 0 (pairwise tree), per block
                    g4 = G[:, : cols * DIN].rearrange(
                        "p (b s f) -> p b s f", s=S, f=DIN
                    )
                    s = S
                    while s > 1:
                        half = s // 2
                        hi0 = s - half
                        nc.vector.tensor_tensor(
                            out=g4[:, :, 0:half, :],
                            in0=g4[:, :, 0:half, :],
                            in1=g4[:, :, hi0:s, :],
                            op=mybir.AluOpType.add,
                        )
                        s = hi0
                    # copy folded result to accumulator columns
                    nc.vector.tensor_copy(
                        out=acc_sb[:, outpos * DIN : (outpos + nbs) * DIN].rearrange(
                            "p (b f) -> p b f", f=DIN
                        ),
                        in_=g4[:, :, 0, :],
                    )

            # ---- realign mo to mi-local order via DRAM scratch ----
            nc.sync.dma_start(
                mo_scratch[:].rearrange("(j p) f -> p j f", p=P),
                mo_sb[:].rearrange("p (j f) -> p j f", f=DIN),
            )
            for c in range(nb):
                nc.gpsimd.indirect_dma_start(
                    out=mo2_sb[:, c * DIN : (c + 1) * DIN],
                    out_offset=None,
                    in_=mo_scratch[:],
                    in_offset=bass.IndirectOffsetOnAxis(
                        ap=realign_sb[:, c : c + 1], axis=0
                    ),
                )

            # ---- MLP over 128-node tiles (mi-local order) ----
            pos_in = meta["lay_in"][1]
            OG = 4  # output tiles per DMA group
            for b0 in range(0, nb, OG):
                og = min(OG, nb - b0)
                os_ = ostp.tile([P, OG * P], f32, tag="os")
                for j in range(og):
                    b = b0 + j
                    mi_c = mi_sb[:, pos_in[b] * DIN : (pos_in[b] + 1) * DIN]
                    mo_c = mo2_sb[:, b * DIN : (b + 1) * DIN]
                    xo_c = xo_sb[:, b * DIN : (b + 1) * DIN]
                    hp = psH.tile([P, P], f32, tag="hp")
                    for q, (src_c, w1q) in enumerate(
                        ((mi_c, w1ta_sb), (mo_c, w1tb_sb), (xo_c, w1tc_sb))
                    ):
                        pA = psA.tile([DIN, P], f32, tag="pA")
                        nc.tensor.transpose(pA[:], src_c, ident[:])
                        mt = mlp.tile([DIN, P], f32, tag="mt")
                        nc.vector.tensor_copy(out=mt[:], in_=pA[:])
                        nc.tensor.matmul(
                            hp[:], w1q[:], mt[:], start=(q == 0), stop=(q == 2)
                        )
                    hs = mlp.tile([P, P], f32, tag="hs")
                    nc.scalar.activation(
                        hs[:],
                        hp[:],
                        mybir.ActivationFunctionType.Tanh,
                        bias=b1_sb[:],
                        scale=1.0,
                    )
                    op_ = psH.tile([P, P], f32, tag="op")
                    nc.tensor.matmul(op_[:], w2t_sb[:], hs[:], start=True, stop=True)
                    nc.scalar.activation(
                        os_[:, j * P : (j + 1) * P],
                        op_[:],
                        mybir.ActivationFunctionType.Tanh,
                        bias=b2_sb[:],
                        scale=1.0,
                    )
                nc.sync.dma_start(
                    out_t[:, b0 * P : (b0 + og) * P], os_[:, : og * P]
                )

    nc.compile()
    return nc


def _S_of(lay, j):
    """Slot count of j-th block (in ordb order)."""
    ordb = lay[0]
    col_off = lay[2]
    total = lay[3]
    nxt = col_off[j + 1] if j + 1 < len(col_off) else total
    return nxt - col_off[j]


def kernel(x, edge_index, edge_attr, W1, b1, W2, b2):
    x = np.asarray(x, np.float32)
    meta = _host_prep(x, edge_index, edge_attr)
    key = (meta["N"], meta["E"], meta["nb"], meta["CM"], meta["CO"],
           tuple(meta["lay_in"][2]), tuple(meta["lay_out"][2]))
    if key not in _PROG_CACHE:
        _PROG_CACHE[key] = _build_program(meta)
    nc = _PROG_CACHE[key]

    W1 = np.asarray(W1, np.float32)
    W2 = np.asarray(W2, np.float32)
    b1 = np.asarray(b1, np.float32).reshape(DHID, 1)
    b2 = np.asarray(b2, np.float32).reshape(DHID, 1)
    w1t = np.ascontiguousarray(W1.T)  # [144, 128]
    w1ta = np.ascontiguousarray(w1t[:DIN])
    w1tb = np.ascontiguousarray(w1t[DIN : 2 * DIN])
    w1tc = np.ascontiguousarray(w1t[2 * DIN :])
    w2t = np.ascontiguousarray(W2.T)

    in_maps = []
    for k in range(NCORES):
        in_maps.append(
            {
                "x_tab": meta["x_tab"],
                "idx_mi": meta["idx_mi"][k],
                "w_mi": meta["w_mi"][k],
                "idx_mo": meta["idx_mo"][k],
                "w_mo": meta["w_mo"][k],
                "idx_own": meta["idx_own"][k],
                "realign": meta["realign"][k],
                "w1ta": w1ta,
                "w1tb": w1tb,
                "w1tc": w1tc,
                "w2t": w2t,
                "b1": b1,
                "b2": b2,
            }
        )

    runner = _get_runner(nc)
    results = runner.run(in_maps)
    global _LAST
    _LAST = (nc, in_maps)

    N = meta["N"]
    out = np.empty((meta["npad"], DHID), np.float32)
    for k in range(NCORES):
        out[meta["nodes_of_core"][k]] = results[k]["out_t"].T
    return out[:N]


_LAST = None
_RUNNER_CACHE: dict = {}


class _PjrtRunner:
    """Builds the shard_map-jitted NEFF executor once; supports repeated
    dispatches with device-resident inputs for timing."""

    def __init__(self, nc):
        import jax
        import jax.numpy as jnp
        import concourse.mybir as mybir
        from concourse import bass2jax
        from jax.sharding import Mesh, NamedSharding, PartitionSpec
        from jax.experimental.shard_map import shard_map

        bass2jax.install_neuronx_cc_hook()
        self.jax = jax
        self.jnp = jnp
        in_names: list[str] = []
        out_names: list[str] = []
        out_avals = []
        out_shapes = []
        partition_name = (
            nc.partition_id_tensor.name if nc.partition_id_tensor else None
        )
        for alloc in nc.m.functions[0].allocations:
            if not isinstance(alloc, mybir.MemoryLocationSet):
                continue
            name = alloc.memorylocations[0].name
            if alloc.kind == "ExternalInput":
                if name != partition_name:
                    in_names.append(name)
            elif alloc.kind == "ExternalOutput":
                shape = tuple(alloc.tensor_shape)
                dtype = mybir.dt.np(alloc.dtype)
                out_names.append(name)
                out_avals.append(jax.core.ShapedArray(shape, dtype))
                out_shapes.append((shape, dtype))
        self.in_names = in_names
        self.out_names = out_names
        self.out_shapes = out_shapes
        n_params = len(in_names)
        n_outs = len(out_names)
        all_names = in_names + out_names
        if partition_name is not None:
            all_names = all_names + [partition_name]

        def _body(*args):
            operands = list(args)
            if partition_name is not None:
                operands.append(bass2jax.partition_id_tensor())
            outs = bass2jax._bass_exec_p.bind(
                *operands,
                out_avals=tuple(out_avals),
                in_names=tuple(all_names),
                out_names=tuple(out_names),
                lowering_input_output_aliases=(),
                sim_require_finite=True,
                sim_require_nnan=True,
                nc=nc,
            )
            return tuple(outs)

        devices = jax.devices()[:NCORES]
        self.mesh = Mesh(np.asarray(devices), ("core",))
        spec = PartitionSpec("core")
        self.sharding = NamedSharding(self.mesh, spec)
        self.sharded = jax.jit(
            shard_map(
                _body,
                mesh=self.mesh,
                in_specs=(spec,) * (n_params + n_outs),
                out_specs=(spec,) * n_outs,
                check_rep=False,
            ),
            donate_argnums=tuple(range(n_params, n_params + n_outs)),
            keep_unused=True,
        )

        def _mk_zeros():
            return tuple(
                jnp.zeros((NCORES * s[0], *s[1:]), d) for s, d in out_shapes
            )

        self.zeros_fn = jax.jit(
            _mk_zeros, out_shardings=(self.sharding,) * n_outs
        )

    def _stage_inputs(self, in_maps):
        concat = [
            np.concatenate([np.asarray(in_maps[c][n]) for c in range(NCORES)], axis=0)
            for n in self.in_names
        ]
        return [self.jax.device_put(a, self.sharding) for a in concat]

    def _dispatch(self, staged):
        zeros = self.zeros_fn()
        outs = self.sharded(*staged, *zeros)
        self.jax.block_until_ready(outs)
        return outs

    def run(self, in_maps):
        staged = self._stage_inputs(in_maps)
        outs = self._dispatch(staged)
        res = []
        for c in range(NCORES):
            m = {}
            for i, n in enumerate(self.out_names):
                s, d = self.out_shapes[i]
                m[n] = np.asarray(outs[i]).reshape(NCORES, *s)[c]
            res.append(m)
        return res

    def timed(self, in_maps, iters=10):
        """Wall-clock per dispatch (tunnel RTT included) minus a tiny-dispatch
        baseline; min-statistics over iters. Noisy through the axon tunnel —
        treat as an upper-bound cross-check of the cost model."""
        import time

        staged = self._stage_inputs(in_maps)
        self._dispatch(staged)  # warm
        walls = []
        for _ in range(iters):
            zeros = self.zeros_fn()
            self.jax.block_until_ready(zeros)
            t0 = time.perf_counter()
            outs = self.sharded(*staged, *zeros)
            self.jax.block_until_ready(outs)
            walls.append(time.perf_counter() - t0)
        tiny = self.jax.device_put(np.zeros((NCORES, 8), np.float32), self.sharding)
        base_fn = self.jax.jit(lambda a: a + 1.0)
        self.jax.block_until_ready(base_fn(tiny))
        bases = []
        for _ in range(iters):
            t0 = time.perf_counter()
            self.jax.block_until_ready(base_fn(tiny))
            bases.append(time.perf_counter() - t0)
        wall, base = min(walls), min(bases)
        print(
            f"kernel walls min/med: {wall*1e3:.2f}/{np.median(walls)*1e3:.2f} ms; "
            f"baseline min/med: {base*1e3:.2f}/{np.median(bases)*1e3:.2f} ms"
        )
        return max(wall - base, 0.0) * 1e9


def _get_runner(nc):
    r = _RUNNER_CACHE.get(id(nc))
    if r is None:
        r = _PjrtRunner(nc)
        _RUNNER_CACHE[id(nc)] = r
    return r


def time_kernel(inputs=None, iters=8):
    """Median wall-clock of a kernel dispatch minus tiny-dispatch baseline, ns.
    Must be called after kernel()."""
    assert _LAST is not None, "call kernel() first"
    nc, in_maps = _LAST
    return _get_runner(nc).timed(in_maps, iters=iters)
